# revision 6
# baseline (speedup 1.0000x reference)
"""Conv2d 3x3 (stride 1, pad 1) as implicit GEMM on 8 Trainium2 NeuronCores.

Problem: x[32,128,56,56] f32, weights[128,128,3,3] f32, bias[128] f32
         -> out[32,128,56,56] f32.

Sharding: data-parallel over batch — 4 images per core, weights/bias
replicated on every core.

Per-core kernel layout:
  - channels (128) live on the SBUF partition dim.
  - each image is stored in a zero-padded flat row layout:
      [margin 59 | 56 rows x 58 (0 | 56 data | 0) | margin 59]
    so every conv tap (kh,kw) is a plain shifted window d=(kh-1)*58+(kw-1)
    of the same flat buffer.
  - output is computed in 7 PSUM groups of 464 (= 8 padded rows) per
    image; each group accumulates 9 matmuls (one per tap, K=128 fp32r).
  - bias is fused into the PSUM->SBUF eviction on the scalar engine,
    which also drops the pad columns.
"""

import numpy as np

N_TOTAL = 32
N_CORES = 8
N_PER_CORE = N_TOTAL // N_CORES
C = 128
H = W = 56
HW = H * W            # 3136
WP = W + 2            # 58  padded row width
L = H * WP            # 3248 flat padded length
MARGIN = WP + 1       # 59  covers worst tap offset
TILE_W = MARGIN + L + MARGIN  # 3366
ROWS_PER_G = 8
GW = ROWS_PER_G * WP  # 464 (<=512 fp32 PSUM bank)
N_GROUPS = H // ROWS_PER_G  # 7
GI = ROWS_PER_G * W   # 448 interior elements per group

_CACHE = {}


def _build_nc():
    import concourse.mybir as mybir
    import concourse.tile as tile
    from concourse import bacc

    f32 = mybir.dt.float32
    f32r = mybir.dt.float32r
    af = mybir.ActivationFunctionType

    nc = bacc.Bacc("TRN2", target_bir_lowering=False, debug=False)

    x_d = nc.dram_tensor("x", [N_PER_CORE, C, HW], f32r, kind="ExternalInput")
    w_d = nc.dram_tensor("w", [C, 9 * C], f32r, kind="ExternalInput")
    b_d = nc.dram_tensor("b", [C, 1], f32, kind="ExternalInput")
    y_d = nc.dram_tensor("y", [N_PER_CORE, C, HW], f32, kind="ExternalOutput")

    with tile.TileContext(nc) as tc:
        with (
            tc.tile_pool(name="const", bufs=1) as cpool,
            tc.tile_pool(name="xbuf", bufs=1) as xpool,
            tc.tile_pool(name="obuf", bufs=2) as opool,
            tc.tile_pool(name="psum", bufs=4, space="PSUM") as ppool,
        ):
            wt = cpool.tile([C, 9 * C], f32r, tag="wt")
            nc.sync.dma_start(out=wt[:], in_=w_d[:])
            bt = cpool.tile([C, 1], f32, tag="bt")
            nc.sync.dma_start(out=bt[:], in_=b_d[:])

            # f32 zero scratch; fp32r padding zeros are written via
            # ScalarE activation copies (f32 -> f32r round) since Memset
            # can't encode fp32r
            zs = cpool.tile([C, 110], f32, tag="zs")
            nc.vector.memset(zs[:], 0.0)

            xts = []
            for n in range(N_PER_CORE):
                xt = xpool.tile([C, TILE_W], f32r, tag=f"x{n}")
                # zero only the positions matmuls read as padding:
                # [left margin + col0 of row 0), (col57 of row r, col0 of
                # row r+1) pairs, (col57 of last row + right margin]
                nc.scalar.activation(
                    xt[:, 0 : MARGIN + 1], zs[:, 0 : MARGIN + 1], af.Copy
                )
                nc.scalar.activation(
                    xt[:, MARGIN + L - 1 : TILE_W],
                    zs[:, 0 : TILE_W - (MARGIN + L - 1)],
                    af.Copy,
                )
                pairs = xt[:, MARGIN + W + 1 : MARGIN + W + 1 + (H - 1) * WP]
                pairs = pairs.rearrange("p (r c) -> p r c", c=WP)[:, :, 0:2]
                zpairs = zs[:, 0 : 2 * (H - 1)].rearrange(
                    "p (r c) -> p r c", c=2
                )
                nc.scalar.activation(pairs, zpairs, af.Copy)
                dst = xt[:, MARGIN + 1 : MARGIN + 1 + L]
                dst = dst.rearrange("p (r c) -> p r c", c=WP)[:, :, 0:W]
                nc.sync.dma_start(
                    out=dst,
                    in_=x_d[n].rearrange("p (r c) -> p r c", c=W),
                )
                xts.append(xt)

            for n in range(N_PER_CORE):
                ot = opool.tile([C, HW], f32, tag="out")
                for g in range(N_GROUPS):
                    ps = ppool.tile([C, GW], f32, tag="ps")
                    for t in range(9):
                        kh, kw = divmod(t, 3)
                        d = (kh - 1) * WP + (kw - 1)
                        base = MARGIN + g * GW + d
                        rhs = xts[n][:, base : base + GW]
                        lhsT = wt[:, t * C : (t + 1) * C]
                        nc.tensor.matmul(
                            ps[:], lhsT, rhs, start=(t == 0), stop=(t == 8)
                        )
                    src = ps[:].rearrange("p (r c) -> p r c", c=WP)[:, :, 1 : W + 1]
                    dstp = ot[:, g * GI : (g + 1) * GI]
                    dstp = dstp.rearrange("p (r c) -> p r c", c=W)
                    nc.scalar.activation(dstp, src, af.Identity, bias=bt[:])
                nc.sync.dma_start(out=y_d[n], in_=ot[:])

    nc.compile()
    return nc


def _get_nc():
    if "nc" not in _CACHE:
        _CACHE["nc"] = _build_nc()
    return _CACHE["nc"]


def _prep_inputs(x, weights, bias):
    x = np.ascontiguousarray(np.asarray(x, dtype=np.float32)).reshape(
        N_TOTAL, C, HW
    )
    # weights [co, ci, kh, kw] -> [ci, (kh kw), co] so each tap slice is a
    # contiguous [K=ci, M=co] lhsT tile
    w = np.asarray(weights, dtype=np.float32)
    w = np.ascontiguousarray(np.transpose(w, (1, 2, 3, 0)).reshape(C, 9 * C))
    b = np.ascontiguousarray(np.asarray(bias, dtype=np.float32).reshape(C, 1))
    return x, w, b


def kernel(x, weights, bias, _trace=False):
    from concourse.bass_utils import run_bass_kernel_spmd

    nc = _get_nc()
    x, w, b = _prep_inputs(x, weights, bias)
    in_maps = [
        {"x": x[i * N_PER_CORE : (i + 1) * N_PER_CORE], "w": w, "b": b}
        for i in range(N_CORES)
    ]
    res = run_bass_kernel_spmd(
        nc, in_maps, core_ids=list(range(N_CORES)), trace=_trace
    )
    y = np.concatenate([r["y"] for r in res.results], axis=0)
    y = y.reshape(N_TOTAL, C, H, W)
    if _trace:
        return y, res
    return y


# revision 12
# speedup vs baseline: 1.1480x; 1.1480x over previous
"""Conv2d 3x3 (stride 1, pad 1) as implicit GEMM on 8 Trainium2 NeuronCores.

Problem: x[32,128,56,56] f32, weights[128,128,3,3] f32, bias[128] f32
         -> out[32,128,56,56] f32.

Sharding: data-parallel over batch — 4 images per core, weights/bias
replicated on every core.

Per-core kernel design:
  - channels (128) live on the SBUF partition dim.
  - the host pre-pads each image into a flat row layout
      [margin 58 | 56 rows x (56 data + 1 zero) | margin 58]
    With data-first rows, the left neighbour of col 0 of row r is row
    r-1's right pad, and the right neighbour of col 55 is row r's own
    pad — so every conv tap (kh,kw) is a plain shifted window
    d=(kh-1)*57+(kw-1) of one flat buffer, and the device DMA is fully
    contiguous (descriptor-cheap).
  - output is computed in 7 PSUM groups of 456 (= 8 padded rows) per
    image; each group accumulates 9 matmuls (one per tap, K=128 fp32r).
  - bias is fused into the PSUM->SBUF eviction on the scalar engine,
    which also drops the pad columns; each group's result is DMA'd out
    immediately so the tail only waits on the last group.
  - a few throwaway matmuls on the weight tile warm the PE clock (HAM)
    while the first image's DMA is in flight.
"""

import numpy as np

N_TOTAL = 32
N_CORES = 8
N_PER_CORE = N_TOTAL // N_CORES
C = 128
H = W = 56
HW = H * W            # 3136
WP = W + 1            # 57  padded row width (shared pad col)
L = H * WP            # 3192 flat padded length
MARGIN = WP + 1       # 58  covers worst tap offset
TILE_W = MARGIN + L + MARGIN  # 3308
ROWS_PER_G = 8
GW = ROWS_PER_G * WP  # 456 (<=512 fp32 PSUM bank)
N_GROUPS = H // ROWS_PER_G  # 7
GI = ROWS_PER_G * W   # 448 interior elements per group
N_WARMUP = 7
X_BOUNDS = [0, MARGIN + GW + MARGIN, MARGIN + 3 * GW + MARGIN,
            MARGIN + 5 * GW + MARGIN, TILE_W]

_CACHE = {}


def _build_nc():
    import concourse.mybir as mybir
    import concourse.tile as tile
    from concourse import bacc

    f32 = mybir.dt.float32
    f32r = mybir.dt.float32r
    af = mybir.ActivationFunctionType

    nc = bacc.Bacc("TRN2", target_bir_lowering=False, debug=False)

    x_d = nc.dram_tensor("x", [N_PER_CORE, C, TILE_W], f32r, kind="ExternalInput")
    w_d = nc.dram_tensor("w", [C, 9 * C], f32r, kind="ExternalInput")
    b_d = nc.dram_tensor("b", [C, 1], f32, kind="ExternalInput")
    y_d = nc.dram_tensor("y", [N_PER_CORE, C, HW], f32, kind="ExternalOutput")

    with tile.TileContext(nc) as tc:
        with (
            tc.tile_pool(name="const", bufs=1) as cpool,
            tc.tile_pool(name="xbuf", bufs=1) as xpool,
            tc.tile_pool(name="obuf", bufs=2) as opool,
            tc.tile_pool(name="psum", bufs=4, space="PSUM") as ppool,
            tc.tile_pool(name="warm", bufs=2, space="PSUM") as wpool,
        ):
            from concourse.tile import add_dep_helper

            # PE warm-up on a zero scratch (fp32, no DMA dependency): the
            # HAM clock gate needs ~3.4us of PE activity before it runs
            # the array at full speed, and these can start right after the
            # framework preamble, fully inside the DMA wait.
            zsc = cpool.tile([C, 256], f32, tag="zsc")
            nc.vector.memset(zsc[:], 0.0)
            for k in range(N_WARMUP):
                wm = wpool.tile([C, 256], f32, tag="wm")
                nc.tensor.matmul(
                    wm[:], zsc[:, 0:C], zsc[:], start=True, stop=True
                )

            # Weights in 3 per-tap-group chunks: tile sub-range dep
            # tracking lets group 0's first taps start while later taps
            # are still in flight.
            wt = cpool.tile([C, 9 * C], f32r, tag="wt")
            for k in range(3):
                nc.sync.dma_start(
                    out=wt[:, k * 3 * C : (k + 1) * 3 * C],
                    in_=w_d[:, k * 3 * C : (k + 1) * 3 * C],
                )
            bt = cpool.tile([C, 1], f32, tag="bt")
            nc.sync.dma_start(out=bt[:], in_=b_d[:])

            xts = []
            x_dmas = {}
            for n in range(N_PER_CORE):
                xt = xpool.tile([C, TILE_W], f32r, tag=f"x{n}")
                for a, b in zip(X_BOUNDS, X_BOUNDS[1:]):
                    x_dmas.setdefault(n, []).append(
                        nc.sync.dma_start(out=xt[:, a:b], in_=x_d[n][:, a:b])
                    )
                xts.append(xt)

            gate_mms = {}
            gate2_mms = {}
            for n in range(N_PER_CORE):
                ot = opool.tile([C, HW], f32, tag="out")
                for g in range(N_GROUPS):
                    ps = ppool.tile([C, GW], f32, tag="ps")
                    for t in range(9):
                        kh, kw = divmod(t, 3)
                        d = (kh - 1) * WP + (kw - 1)
                        base = MARGIN + g * GW + d
                        rhs = xts[n][:, base : base + GW]
                        lhsT = wt[:, t * C : (t + 1) * C]
                        mm = nc.tensor.matmul(
                            ps[:], lhsT, rhs, start=(t == 0), stop=(t == 8)
                        )
                        if g == 0 and t == 8:
                            gate_mms[n] = mm
                        if g == 2 and t == 8:
                            gate2_mms[n] = mm
                    src = ps[:].rearrange("p (r c) -> p r c", c=WP)[:, :, 0:W]
                    dstp = ot[:, g * GI : (g + 1) * GI]
                    dstp = dstp.rearrange("p (r c) -> p r c", c=W)
                    nc.scalar.activation(dstp, src, af.Identity, bias=bt[:])
                    nc.sync.dma_start(
                        out=y_d[n][:, g * GI : (g + 1) * GI],
                        in_=ot[:, g * GI : (g + 1) * GI],
                    )

            # Keep early HBM bandwidth for w + x0's head: x0's tail
            # chunks start once group 0 is running, and later images'
            # transfers once the previous image is 3 groups in (compute
            # sems are cheap; no HBM-receipt latency).
            for dma in x_dmas[0][2:]:
                add_dep_helper(
                    dma.ins, gate_mms[0].ins, sync=True,
                    reason="stagger x0 tail",
                )
            for n in range(1, N_PER_CORE):
                for dma in x_dmas[n]:
                    add_dep_helper(
                        dma.ins, gate2_mms[n - 1].ins, sync=True,
                        reason="stagger x prefetch",
                    )

    nc.compile()
    return nc


def _get_nc():
    if "nc" not in _CACHE:
        _CACHE["nc"] = _build_nc()
    return _CACHE["nc"]


def _prep_inputs(x, weights, bias):
    x = np.asarray(x, dtype=np.float32).reshape(N_TOTAL, C, H, W)
    xp = np.zeros((N_TOTAL, C, TILE_W), dtype=np.float32)
    # interior: rows of [56 data | 0], flat at offset MARGIN
    v = xp[:, :, MARGIN : MARGIN + L].reshape(N_TOTAL, C, H, WP)
    v[:, :, :, 0:W] = x
    # weights [co, ci, kh, kw] -> [ci, (kh kw), co] so each tap slice is a
    # contiguous [K=ci, M=co] lhsT tile
    w = np.asarray(weights, dtype=np.float32)
    w = np.ascontiguousarray(np.transpose(w, (1, 2, 3, 0)).reshape(C, 9 * C))
    b = np.ascontiguousarray(np.asarray(bias, dtype=np.float32).reshape(C, 1))
    return xp, w, b


def kernel(x, weights, bias, _trace=False):
    from concourse.bass_utils import run_bass_kernel_spmd

    nc = _get_nc()
    xp, w, b = _prep_inputs(x, weights, bias)
    in_maps = [
        {"x": xp[i * N_PER_CORE : (i + 1) * N_PER_CORE], "w": w, "b": b}
        for i in range(N_CORES)
    ]
    res = run_bass_kernel_spmd(
        nc, in_maps, core_ids=list(range(N_CORES)), trace=_trace
    )
    y = np.concatenate([r["y"] for r in res.results], axis=0)
    y = y.reshape(N_TOTAL, C, H, W)
    if _trace:
        return y, res
    return y
